# revision 1
# baseline (speedup 1.0000x reference)
"""Masked multi-head self-attention (sparse_attention) on 8 Trainium2 cores.

Strategy
--------
Shard the fused (batch*heads)=16 leading dim of q/k/v across 8 cores, 2 heads
per core.  Per head the kernel computes S^T = K @ Q^T in [j, i] orientation
(128-row j-chunks on partitions, 512-col i-blocks on the free dim), applies
exp on the scalar engine (no max-subtraction needed: |s*scale| <= ~7 so exp
cannot overflow in fp32, and blocked entries are handled structurally, not
additively), then accumulates O^T = V~^T @ P^T on the tensor engine where
V~ = [V | 1] so the softmax denominators fall out of the same matmuls.

The bbox mask has rank-structure: blocked(i,j) <=> (i in A-only and j in
B-only) or vice versa, where A/B are the two subject boxes.  The host sorts
the j (key/value) axis into [A-only | B-only | rest] with 64-aligned zero
padding, so every 64-row half-chunk belongs to one group.  PV matmuls
accumulate into one PSUM accumulator per group; the final combine applies the
per-i 0/1 weights (wA, wB) and sums the three accumulators — the mask costs
no elementwise work on the n*n tiles at all.  Finally each [81, 512] combined
block is PE-transposed back to [128(i), 81], normalized by the sums column,
and DMA'd out in natural i order (the i/query axis is never permuted).
"""

import math
import os

import numpy as np

N_CORES = 8
P = 128  # partitions / j-chunk rows
IB = 512  # i-block width (psum bank, fp32)
DH = 80  # head dim
SUM_ROW = 96  # 32-aligned partition for the sums row (DVE slice rule)
DV = SUM_ROW + 1  # V padded to 96, plus the ones column

_PROGRAM_CACHE = {}
LAST_RESULTS = None  # BassKernelResults of the most recent run (for test.py)


# ----------------------------------------------------------------------------
# host-side mask analysis (mirrors reference._subject_masks / _self_mask)
# ----------------------------------------------------------------------------

def _subject_masks_np(bboxes: np.ndarray, resolution: int) -> np.ndarray:
    b = bboxes[0].astype(np.float32)  # [s, 4]
    x0 = np.round(b[:, 0] * resolution)
    y0 = np.round(b[:, 1] * resolution)
    x1 = np.round(b[:, 2] * resolution)
    y1 = np.round(b[:, 3] * resolution)
    coords = np.arange(resolution, dtype=np.float32)
    xm = (coords[None, :] >= x0[:, None]) & (coords[None, :] < x1[:, None])
    ym = (coords[None, :] >= y0[:, None]) & (coords[None, :] < y1[:, None])
    return (ym[:, :, None] & xm[:, None, :]).reshape(b.shape[0], -1)  # [s, n]


def _group_layout(bboxes: np.ndarray, n: int):
    """Sort the j axis into [A-only | B-only | rest], 64-aligned groups.

    Returns (perm, seg_sizes, group_starts, n_pad, wA, wB) where perm is the
    source index for each padded slot (-1 for zero padding), wA/wB are the
    per-original-i {0,1} combine weights for the A/B accumulators.
    """
    res = int(math.isqrt(n))
    assert res * res == n
    subj = _subject_masks_np(bboxes, res)
    assert subj.shape[0] == 2, "kernel specialized for 2 subject boxes"
    m0, m1 = subj[0], subj[1]
    e0 = m0 & ~m1  # A-only
    e1 = m1 & ~m0  # B-only
    rest = ~(e0 | e1)

    idx = np.arange(n)
    groups = [idx[e0], idx[e1], idx[rest]]

    def ceil64(x):
        return ((x + 63) // 64) * 64

    padded = [ceil64(len(g)) for g in groups]
    n_pad = sum(padded)
    if n_pad % P:
        padded[2] += 64
        n_pad += 64
    perm = np.full(n_pad, -1, dtype=np.int64)
    starts = []
    pos = 0
    for g, plen in zip(groups, padded):
        starts.append(pos)
        perm[pos : pos + len(g)] = g
        pos += plen

    wA = (~e1).astype(np.float32)  # zero the A accumulator for i in B-only
    wB = (~e0).astype(np.float32)  # zero the B accumulator for i in A-only
    return perm, padded, starts, n_pad, wA, wB


def _chunk_segments(padded, starts, n_pad):
    """Per 128-chunk: list of (row_lo, row_hi, group_id) 64-aligned segments."""
    half_group = np.empty(n_pad // 64, dtype=np.int64)
    for gid, (st, plen) in enumerate(zip(starts, padded)):
        half_group[st // 64 : (st + plen) // 64] = gid
    segments = []
    for c in range(n_pad // P):
        g0 = int(half_group[2 * c])
        g1 = int(half_group[2 * c + 1])
        if g0 == g1:
            segments.append([(0, P, g0)])
        else:
            segments.append([(0, 64, g0), (64, P, g1)])
    return segments


# ----------------------------------------------------------------------------
# device program
# ----------------------------------------------------------------------------

def _build_program(n, n_pad, heads_per_core, segments, present_groups, scale):
    import concourse.mybir as mybir
    import concourse.tile as tile
    from concourse import bacc

    f32 = mybir.dt.float32
    f32r = mybir.dt.float32r
    nch = n_pad // P
    n_ib = n // IB
    Exp = mybir.ActivationFunctionType.Exp
    MUL = mybir.AluOpType.mult
    ADD = mybir.AluOpType.add

    nc = bacc.Bacc("TRN2", target_bir_lowering=False, debug=False,
                   num_devices=N_CORES)
    qT_d = nc.dram_tensor("qT", [heads_per_core, DH, n], f32r, kind="ExternalInput")
    kT_d = nc.dram_tensor("kT", [heads_per_core, DH, n_pad], f32r,
                          kind="ExternalInput")
    vt_d = nc.dram_tensor("vt", [heads_per_core, n_pad, DV], f32r,
                          kind="ExternalInput")
    wab_d = nc.dram_tensor("wab", [1, n], f32, kind="ExternalInput")
    wbb_d = nc.dram_tensor("wbb", [1, n], f32, kind="ExternalInput")
    id_d = nc.dram_tensor("ident", [P, P], f32, kind="ExternalInput")
    o_d = nc.dram_tensor("o", [heads_per_core, n, DH], f32,
                         kind="ExternalOutput")

    # Chunk processing order: a pure-neutral singleton first (short
    # cross-i-block dependency chain), then every chunk touching the A/B
    # accumulators (so those accumulators finish early and the combine
    # overlaps the long neutral tail), then the remaining neutral chunks.
    ab_chunks = [c for c, segs in enumerate(segments)
                 if any(g != 2 for (_, _, g) in segs)]
    n_chunks = [c for c in range(nch) if c not in ab_chunks]
    ab_rest = list(ab_chunks)
    order = []
    if nch % 2:
        order.append((ab_rest.pop(0),) if ab_rest else (n_chunks.pop(),))
    rest = ab_rest + n_chunks
    order += [tuple(rest[i : i + 2]) for i in range(0, len(rest), 2)]
    last_ab_pair = max((t for t, pr in enumerate(order)
                        if any(c in ab_chunks for c in pr)), default=None)

    # first/last (chunk, row) PV matmul per group in traversal order
    first_seg = {}
    last_seg = {}
    for pr in order:
        for c in pr:
            for (r0, _, g) in segments[c]:
                first_seg.setdefault(g, (c, r0))
                last_seg[g] = (c, r0)

    with tile.TileContext(nc) as tc:
        with (
            tc.tile_pool(name="const", bufs=1) as const_pool,
            tc.tile_pool(name="head", bufs=2) as head_pool,
            tc.tile_pool(name="p", bufs=3) as p_pool,
            tc.tile_pool(name="comb", bufs=2) as comb_pool,
            tc.tile_pool(name="out", bufs=4) as out_pool,
            tc.tile_pool(name="s_ps", bufs=2, space="PSUM") as s_pool,
            tc.tile_pool(name="acc_ps", bufs=1, space="PSUM") as acc_pool,
            tc.tile_pool(name="tr_ps", bufs=1, space="PSUM") as tr_pool,
        ):
            wab_t = const_pool.tile([DV, n], f32)
            wbb_t = const_pool.tile([DV, n], f32)
            ident = const_pool.tile([P, P], f32)
            nc.sync.dma_start(ident[:], id_d[:])

            # pre-warm the exp table set while the first DMAs run
            warm = const_pool.tile([P, 1], f32)
            nc.vector.memset(warm[:], 0.0)
            nc.scalar.activation(warm[:], warm[:], Exp)

            def load_head(h, eng=None):
                eng = eng or nc.sync
                kT_t = head_pool.tile([DH, nch, P], f32r, tag="kT",
                                      name=f"kT_{h}")
                qT_t = head_pool.tile([DH, n], f32r, tag="qT", name=f"qT_{h}")
                vt_t = head_pool.tile([P, nch, DV], f32r, tag="vt",
                                      name=f"vt_{h}")
                kT_src = kT_d[h].rearrange("d (c j) -> d c j", j=P)
                vt_src = vt_d[h].rearrange("(c p) d -> p c d", p=P)
                # traversal-ordered slices; first slice covers the first pairs
                lead = sorted(set(order[0] + order[1]))
                hi = max(lead) + 1
                cuts = [0, hi]
                for c in (hi + 4, hi + 10, hi + 18, nch):
                    if c > cuts[-1] and c <= nch:
                        cuts.append(min(c, nch))
                if cuts[-1] != nch:
                    cuts.append(nch)
                eng.dma_start(kT_t[:, 0:hi, :], kT_src[:, 0:hi, :])
                eng.dma_start(qT_t[:, 0:IB], qT_d[h][:, 0:IB])
                eng.dma_start(vt_t[:, 0:hi, :], vt_src[:, 0:hi, :])
                ib_next = 1
                for c0, c1 in zip(cuts[1:], cuts[2:]):
                    eng.dma_start(kT_t[:, c0:c1, :], kT_src[:, c0:c1, :])
                    eng.dma_start(vt_t[:, c0:c1, :], vt_src[:, c0:c1, :])
                    if ib_next < n_ib:
                        nc.sync.dma_start(
                            qT_t[:, ib_next * IB : (ib_next + 1) * IB],
                            qT_d[h][:, ib_next * IB : (ib_next + 1) * IB])
                        ib_next += 1
                for ib2 in range(ib_next, n_ib):
                    eng.dma_start(qT_t[:, ib2 * IB : (ib2 + 1) * IB],
                                      qT_d[h][:, ib2 * IB : (ib2 + 1) * IB])
                return kT_t, qT_t, vt_t

            # weight rows first: tiny DMAs into the idle pre-compute
            # window, then replicate across DV partitions via the DMA
            # broadcast path (no PE/PSUM involvement)
            wa_row = const_pool.tile([1, n], f32)
            nc.sync.dma_start(wa_row[:], wab_d[:])
            wb_row = const_pool.tile([1, n], f32)
            nc.sync.dma_start(wb_row[:], wbb_d[:])
            nc.gpsimd.partition_broadcast(wab_t[:], wa_row[:], channels=DV)
            nc.gpsimd.partition_broadcast(wbb_t[:], wb_row[:], channels=DV)
            head_tiles = {0: load_head(0)}

            pending_epilogue = None
            pending_epilogue_b = None
            pending_pv = None
            consts_loaded = [False]

            for h in range(heads_per_core):
                if h not in head_tiles:
                    head_tiles[h] = load_head(h)
                kT_t, qT_t, vt_t = head_tiles[h]

                for ib in range(n_ib):
                    accs = {
                        g: acc_pool.tile([DV, IB], f32, tag=f"acc{g}",
                                         name=f"acc{g}_{h}_{ib}")
                        for g in present_groups
                    }
                    cell = {}
                    q_sl = qT_t[:, ib * IB : (ib + 1) * IB]

                    def make_partial(accs=accs, h=h, ib=ib, cell=cell):
                        def partial():
                            # A/B accumulators are final: fold them with the
                            # per-i weights now, overlapping the neutral tail
                            i_sl = slice(ib * IB, (ib + 1) * IB)
                            t12 = None
                            if 0 in accs:
                                t1 = comb_pool.tile([DV, IB], f32, tag="t1",
                                                    name=f"t1_{h}_{ib}")
                                nc.vector.tensor_tensor(
                                    t1[:], accs[0][:], wab_t[:, i_sl], op=MUL)
                                t12 = t1
                            if 1 in accs:
                                t2 = comb_pool.tile([DV, IB], f32, tag="t2",
                                                    name=f"t2_{h}_{ib}")
                                nc.vector.tensor_tensor(
                                    t2[:], accs[1][:], wbb_t[:, i_sl], op=MUL)
                                if t12 is None:
                                    t12 = t2
                                else:
                                    nc.vector.tensor_tensor(t12[:], t12[:],
                                                            t2[:], op=ADD)
                            cell["t12"] = t12
                        return partial

                    pending_partial = (make_partial()
                                       if last_ab_pair is not None else None)
                    if pending_partial is None:
                        cell["t12"] = None

                    for t, pr in enumerate(order):
                        s_t = s_pool.tile([P, IB * len(pr)], f32, tag="s")
                        for pi, c in enumerate(pr):
                            nc.tensor.matmul(
                                s_t[:, pi * IB : (pi + 1) * IB],
                                lhsT=kT_t[:, c, :],
                                rhs=q_sl,
                                start=True,
                                stop=True,
                            )
                        p_t = p_pool.tile([P, IB * len(pr)], f32r, tag="p")
                        nc.scalar.activation(p_t[:], s_t[:], Exp, scale=scale)
                        if pending_pv is not None:
                            pending_pv()
                            pending_pv = None
                            if (pending_partial is not None
                                    and t == last_ab_pair + 1):
                                pending_partial()
                                pending_partial = None
                        if t == 2 and pending_epilogue is not None:
                            pending_epilogue()
                            pending_epilogue = None
                        elif t == 4 and pending_epilogue_b is not None:
                            pending_epilogue_b()
                            pending_epilogue_b = None

                        def make_pv(pr=pr, p_t=p_t, accs=accs, vt_t=vt_t):
                            def pv():
                                for pi, c in enumerate(pr):
                                    for (r0, r1, g) in segments[c]:
                                        nc.tensor.matmul(
                                            accs[g][:],
                                            lhsT=vt_t[r0:r1, c, :],
                                            rhs=p_t[r0:r1,
                                                    pi * IB : (pi + 1) * IB],
                                            start=((c, r0) == first_seg[g]),
                                            stop=((c, r0) == last_seg[g]),
                                        )
                            return pv

                        pending_pv = make_pv()

                    if pending_partial is not None:
                        # A/B tail reached the end of the block; flush the lag
                        pending_pv()
                        pending_pv = None
                        pending_partial()
                        pending_partial = None

                    def make_epilogue_a(accs=accs, h=h, ib=ib, cell=cell):
                        def epilogue_a():
                            comb = comb_pool.tile([DV, IB], f32, tag="comb",
                                                  name=f"comb_{h}_{ib}")
                            t12 = cell["t12"]
                            i_sl = slice(ib * IB, (ib + 1) * IB)
                            if t12 is None and (0 in accs or 1 in accs):
                                t1 = comb_pool.tile([DV, IB], f32, tag="t1",
                                                    name=f"t1f_{h}_{ib}")
                                parts = []
                                if 0 in accs:
                                    nc.vector.tensor_tensor(
                                        t1[:], accs[0][:], wab_t[:, i_sl],
                                        op=MUL)
                                    parts.append(t1)
                                if 1 in accs:
                                    t2 = comb_pool.tile(
                                        [DV, IB], f32, tag="t2",
                                        name=f"t2f_{h}_{ib}")
                                    nc.vector.tensor_tensor(
                                        t2[:], accs[1][:], wbb_t[:, i_sl],
                                        op=MUL)
                                    if parts:
                                        nc.vector.tensor_tensor(
                                            t1[:], t1[:], t2[:], op=ADD)
                                    else:
                                        parts.append(t2)
                                        t1 = t2
                                nc.vector.tensor_tensor(comb[:], t1[:],
                                                        accs[2][:], op=ADD)
                            elif t12 is not None:
                                nc.vector.tensor_tensor(comb[:], t12[:],
                                                        accs[2][:], op=ADD)
                            else:
                                nc.vector.tensor_copy(comb[:], accs[2][:])
                            cell["comb"] = comb
                        return epilogue_a

                    def make_epilogue_b(h=h, ib=ib, cell=cell):
                        def epilogue_b():
                            comb = cell["comb"]
                            for qq in range(IB // P):
                                tr = tr_pool.tile([P, DV], f32, tag="tr",
                                                  name=f"tr_{h}_{ib}_{qq}")
                                nc.tensor.transpose(
                                    tr[:],
                                    comb[:, qq * P : (qq + 1) * P],
                                    ident[:DV, :DV],
                                )
                                rec = out_pool.tile([P, 1], f32, tag="rec",
                                                    name=f"rec_{h}_{ib}_{qq}")
                                nc.vector.reciprocal(
                                    rec[:], tr[:, SUM_ROW : SUM_ROW + 1])
                                o_t = out_pool.tile([P, DH], f32, tag="o",
                                                    name=f"o_{h}_{ib}_{qq}")
                                nc.vector.tensor_scalar_mul(
                                    o_t[:], tr[:, :DH], rec[:])
                                r0 = ib * IB + qq * P
                                nc.sync.dma_start(o_d[h, r0 : r0 + P, :],
                                                  o_t[:])
                        return epilogue_b

                    # flush leftovers (only reachable when pairs-per-block
                    # is small, e.g. tiny-n debug configs)
                    if pending_epilogue is not None:
                        pending_epilogue()
                    if pending_epilogue_b is not None:
                        pending_epilogue_b()
                    pending_epilogue = make_epilogue_a()
                    pending_epilogue_b = make_epilogue_b()

            if pending_pv is not None:
                pending_pv()
            if pending_epilogue is not None:
                pending_epilogue()
            if pending_epilogue_b is not None:
                pending_epilogue_b()

    nc.compile()
    return nc


# ----------------------------------------------------------------------------
# entry point
# ----------------------------------------------------------------------------

def kernel(hidden_states, q, k, v, bboxes, is_cross, ith, num_heads):
    global LAST_RESULTS
    if is_cross:
        return np.asarray(hidden_states)

    from concourse.bass_utils import run_bass_kernel_spmd

    q = np.ascontiguousarray(np.asarray(q, dtype=np.float32))
    k = np.ascontiguousarray(np.asarray(k, dtype=np.float32))
    v = np.ascontiguousarray(np.asarray(v, dtype=np.float32))
    bboxes = np.asarray(bboxes, dtype=np.float32)
    num_heads = int(num_heads)

    bh, n, dh = q.shape
    assert dh == DH and bh % N_CORES == 0 and n % IB == 0
    heads_per_core = bh // N_CORES
    batch = bh // num_heads
    scale = float(1.0 / np.sqrt(np.float32(dh)))

    perm, padded, starts, n_pad, wA, wB = _group_layout(bboxes, n)
    segments = _chunk_segments(padded, starts, n_pad)
    present_groups = sorted({g for segs in segments for (_, _, g) in segs})

    key = (n, n_pad, heads_per_core, tuple(tuple(s) for s in segments))
    if key not in _PROGRAM_CACHE:
        _PROGRAM_CACHE[key] = _build_program(
            n, n_pad, heads_per_core, segments, present_groups, scale
        )
    nc = _PROGRAM_CACHE[key]

    # host-side input prep
    sel = perm >= 0
    kp = np.zeros((bh, n_pad, dh), np.float32)
    kp[:, sel, :] = k[:, perm[sel], :]
    vt = np.zeros((bh, n_pad, DV), np.float32)
    vt[:, sel, :dh] = v[:, perm[sel], :]
    vt[:, sel, SUM_ROW] = 1.0
    kT = np.ascontiguousarray(kp.transpose(0, 2, 1))  # [bh, dh, n_pad]
    qT = np.ascontiguousarray(q.transpose(0, 2, 1))  # [bh, dh, n]
    wab = np.ascontiguousarray(wA[None, :])
    wbb = np.ascontiguousarray(wB[None, :])

    in_maps = []
    for c in range(N_CORES):
        sl = slice(c * heads_per_core, (c + 1) * heads_per_core)
        in_maps.append({
            "qT": qT[sl], "kT": kT[sl], "vt": vt[sl],
            "wab": wab, "wbb": wbb, "ident": np.eye(P, dtype=np.float32),
        })

    trace = bool(int(os.environ.get("BASS_ATTN_TRACE", "0")))
    kwargs = {}
    if trace:
        kwargs = dict(trace=True, trace_cores=list(range(N_CORES)))
    res = run_bass_kernel_spmd(nc, in_maps, core_ids=list(range(N_CORES)), **kwargs)
    LAST_RESULTS = res

    out = np.empty((batch, n, num_heads * dh), np.float32)
    for bh_idx in range(bh):
        c, hh = divmod(bh_idx, heads_per_core)
        b, hd = divmod(bh_idx, num_heads)
        out[b, :, hd * dh : (hd + 1) * dh] = res.results[c]["o"][hh]
    return out



# revision 3
# speedup vs baseline: 1.2694x; 1.2694x over previous
"""Masked multi-head self-attention (sparse_attention) on 8 Trainium2 cores.

Strategy (v3)
-------------
Shard the fused (batch*heads)=16 leading dim of q/k/v across 8 cores, 2 heads
per core.  Per head the kernel computes S^T = K''@Q''^T in [j, i] orientation
on the tensor engine, where Q''/K'' carry two extra contraction rows that
encode the bbox mask additively: q''[80]=M*mA_i, q''[81]=M*mB_i and
k''[80]=-M*mB_j, k''[81]=-M*mA_j, so blocked (i,j) pairs get -M^2 added to
the score and fall out of both exp paths naturally.  No key sorting, no
accumulator groups, no combine pass.

Scores are produced pre-scaled into fp16-Schraudolph bit space:
t = A_h*u where u = q.k/sqrt(dh) and A_h = 1024/ln2.  The exp(u - C) of each
[128, 1024] score pair-tile is then evaluated on ONE of TWO engines in
parallel (static assignment):
  - ACT pairs (9/16): scalar-engine exp (scale=1/A_h, bias=-C) -> fp16 P.
  - DVE pairs (7/16): one vector-engine tensor_scalar (add IMM, max 0) ->
    int16 whose bit pattern IS fp16(exp(u-C)) (Schraudolph, ~3% rel err).
Both feed plain fp16 PV matmuls accumulating into one [112, 512] PSUM tile
per i-block; softmax denominators fall out of a ones-column in V.  Per
i-block the accumulator is copied to SBUF and DMA'd out unnormalized; the
host divides by the sums row, transposes, and reassembles heads.  Inputs
stream on two DMA queues (sync: q/k, gpsimd: v + outputs).
"""

import math
import os

import numpy as np

N_CORES = 8
P = 128  # partitions / j-chunk rows
IB = 512  # i-block width (psum bank, fp32)
DH = 80  # head dim
DV = 112  # padded V cols
SUM_ROW = 96
MASK = 192.0  # mask row magnitude; blocked scores get -MASK^2
C_SHIFT = 4.0  # global exp shift (range headroom; cancels in softmax)
MU = 0.044  # Schraudolph bias tuning
A_H = 1024.0 / math.log(2.0)
IMM = 15360.0 - 1024.0 * MU - C_SHIFT * A_H

# pair-tile indices handled by the vector engine (rest go to scalar engine)
DVE_PAIRS = (1, 3, 5, 8, 10, 12, 14)

_PROGRAM_CACHE = {}
LAST_RESULTS = None  # BassKernelResults of the most recent run (for test.py)


def _subject_masks_np(bboxes: np.ndarray, resolution: int) -> np.ndarray:
    b = bboxes[0].astype(np.float32)  # [s, 4]
    x0 = np.round(b[:, 0] * resolution)
    y0 = np.round(b[:, 1] * resolution)
    x1 = np.round(b[:, 2] * resolution)
    y1 = np.round(b[:, 3] * resolution)
    coords = np.arange(resolution, dtype=np.float32)
    xm = (coords[None, :] >= x0[:, None]) & (coords[None, :] < x1[:, None])
    ym = (coords[None, :] >= y0[:, None]) & (coords[None, :] < y1[:, None])
    return (ym[:, :, None] & xm[:, None, :]).reshape(b.shape[0], -1)  # [s, n]


def _build_program(n, heads_per_core, dve_pairs):
    import concourse.mybir as mybir
    import concourse.tile as tile
    from concourse import bacc

    f32 = mybir.dt.float32
    f16 = mybir.dt.float16
    i16 = mybir.dt.int16
    Exp = mybir.ActivationFunctionType.Exp
    ADD = mybir.AluOpType.add
    MAX = mybir.AluOpType.max

    nch = n // P
    npair = nch // 2
    n_ib = n // IB

    nc = bacc.Bacc("TRN2", target_bir_lowering=False, debug=False,
                   num_devices=N_CORES)
    q_d = nc.dram_tensor("q16", [heads_per_core, 82, n], f16,
                         kind="ExternalInput")
    k_d = nc.dram_tensor("k16", [heads_per_core, 82, nch, P], f16,
                         kind="ExternalInput")
    v16_d = nc.dram_tensor("v16", [heads_per_core, P, nch, DV], f16,
                           kind="ExternalInput")
    o_d = nc.dram_tensor("o", [heads_per_core, n_ib, 97, IB], f32,
                         kind="ExternalOutput")

    with tile.TileContext(nc) as tc:
        with (
            tc.tile_pool(name="const", bufs=1) as const_pool,
            tc.tile_pool(name="head", bufs=2) as head_pool,
            tc.tile_pool(name="p16", bufs=4) as p16_pool,
            tc.tile_pool(name="pb", bufs=4) as pb_pool,
            tc.tile_pool(name="out", bufs=3) as out_pool,
            tc.tile_pool(name="s_ps", bufs=3, space="PSUM") as s_pool,
            tc.tile_pool(name="acc_ps", bufs=2, space="PSUM") as acc_pool,
        ):
            bias_c = const_pool.tile([P, 1], f32)
            nc.vector.memset(bias_c[:], -C_SHIFT)

            # pre-warm the exp table set while the first DMAs run
            warm = const_pool.tile([P, 1], f32)
            nc.vector.memset(warm[:], 0.0)
            nc.scalar.activation(warm[:], warm[:], Exp)

            def load_head(h):
                q16 = head_pool.tile([82, n], f16, tag="q16", name=f"q16_{h}")
                k16 = head_pool.tile([82, nch, P], f16, tag="k16",
                                     name=f"k16_{h}")
                v16 = head_pool.tile([P, nch, DV], f16, tag="v16",
                                     name=f"v16_{h}")
                # sync queue: q + k, first i-block's needs first
                nc.sync.dma_start(q16[:, 0:IB], q_d[h][:, 0:IB])
                nc.sync.dma_start(k16[:, 0:8, :], k_d[h][:, 0:8, :])
                nc.sync.dma_start(k16[:, 8:20, :], k_d[h][:, 8:20, :])
                nc.sync.dma_start(k16[:, 20:nch, :], k_d[h][:, 20:nch, :])
                nc.sync.dma_start(q16[:, IB:n], q_d[h][:, IB:n])
                # gpsimd queue: v
                nc.gpsimd.dma_start(v16[:, 0:8, :], v16_d[h][:, 0:8, :])
                nc.gpsimd.dma_start(v16[:, 8:20, :], v16_d[h][:, 8:20, :])
                nc.gpsimd.dma_start(v16[:, 20:nch, :], v16_d[h][:, 20:nch, :])
                return q16, k16, v16

            head_tiles = {0: load_head(0)}
            pending_pv = None
            pending_out = None

            for h in range(heads_per_core):
                if h not in head_tiles:
                    head_tiles[h] = load_head(h)
                if h + 1 < heads_per_core and h + 1 not in head_tiles:
                    head_tiles[h + 1] = load_head(h + 1)
                q16, k16, v16 = head_tiles[h]

                for ib in range(n_ib):
                    acc = acc_pool.tile([DV, IB], f32, tag="acc",
                                        name=f"acc_{h}_{ib}")
                    q_sl = q16[:, ib * IB:(ib + 1) * IB]
                    first = [True]

                    for t in range(npair):
                        c0, c1 = 2 * t, 2 * t + 1
                        s = s_pool.tile([P, 2 * IB], f32, tag="s")
                        nc.tensor.matmul(s[:, 0:IB], lhsT=k16[:, c0, :],
                                         rhs=q_sl, start=True, stop=True)
                        nc.tensor.matmul(s[:, IB:2 * IB], lhsT=k16[:, c1, :],
                                         rhs=q_sl, start=True, stop=True)

                        if t in dve_pairs:
                            pb = pb_pool.tile([P, 2 * IB], i16, tag="pb")
                            nc.vector.tensor_scalar(pb[:], s[:], IMM, 0.0,
                                                    op0=ADD, op1=MAX)
                            rhs0 = pb[:, 0:IB].bitcast(f16)
                            rhs1 = pb[:, IB:2 * IB].bitcast(f16)
                        else:
                            p16 = p16_pool.tile([P, 2 * IB], f16, tag="p16")
                            nc.scalar.activation(p16[:], s[:], Exp,
                                                 scale=float(1.0 / A_H),
                                                 bias=bias_c[:])
                            rhs0 = p16[:, 0:IB]
                            rhs1 = p16[:, IB:2 * IB]

                        def make_pv(rhs0=rhs0, rhs1=rhs1, c0=c0, c1=c1, t=t,
                                    acc=acc, v16=v16, first=first):
                            def pv():
                                nc.tensor.matmul(
                                    acc[:], lhsT=v16[:, c0, :], rhs=rhs0,
                                    start=first[0], stop=False)
                                first[0] = False
                                nc.tensor.matmul(
                                    acc[:], lhsT=v16[:, c1, :], rhs=rhs1,
                                    start=False, stop=(t == npair - 1))
                            return pv

                        if pending_pv is not None:
                            pending_pv()
                        pending_pv = make_pv()
                        if t == 2 and pending_out is not None:
                            pending_out()
                            pending_out = None

                    def make_out(acc=acc, h=h, ib=ib):
                        def out():
                            o_sb = out_pool.tile([97, IB], f32, tag="osb",
                                                 name=f"o_{h}_{ib}")
                            nc.vector.tensor_copy(o_sb[:], acc[0:97, :])
                            nc.gpsimd.dma_start(o_d[h, ib], o_sb[:])
                        return out

                    if pending_out is not None:
                        pending_out()
                    pending_out = make_out()

            if pending_pv is not None:
                pending_pv()
            if pending_out is not None:
                pending_out()

    nc.compile()
    return nc


def kernel(hidden_states, q, k, v, bboxes, is_cross, ith, num_heads):
    global LAST_RESULTS
    if is_cross:
        return np.asarray(hidden_states)

    from concourse.bass_utils import run_bass_kernel_spmd

    q = np.asarray(q, dtype=np.float32)
    k = np.asarray(k, dtype=np.float32)
    v = np.asarray(v, dtype=np.float32)
    bboxes = np.asarray(bboxes, dtype=np.float32)
    num_heads = int(num_heads)

    bh, n, dh = q.shape
    assert dh == DH and bh % N_CORES == 0 and n % IB == 0
    heads_per_core = bh // N_CORES
    batch = bh // num_heads
    nch = n // P
    n_ib = n // IB
    scale = 1.0 / math.sqrt(dh)
    g = math.sqrt(A_H * scale)

    res_sq = int(math.isqrt(n))
    subj = _subject_masks_np(bboxes, res_sq)
    assert subj.shape[0] == 2, "kernel specialized for 2 subject boxes"
    mA = (subj[0] & ~subj[1]).astype(np.float32)  # A-only
    mB = (subj[1] & ~subj[0]).astype(np.float32)  # B-only

    f16 = np.float16

    qT = q.transpose(0, 2, 1) * g  # [bh, 80, n]
    kT = k.transpose(0, 2, 1) * g
    q16 = np.empty((bh, 82, n), f16)
    q16[:, :DH] = qT
    q16[:, DH] = MASK * mA
    q16[:, DH + 1] = MASK * mB
    k16 = np.empty((bh, 82, n), f16)
    k16[:, :DH] = kT
    k16[:, DH] = -MASK * mB
    k16[:, DH + 1] = -MASK * mA
    k16 = k16.reshape(bh, 82, nch, P)

    vt = np.zeros((bh, n, DV), f16)
    vt[:, :, :DH] = v
    vt[:, :, SUM_ROW] = 1.0
    v16 = np.ascontiguousarray(
        vt.reshape(bh, nch, P, DV).transpose(0, 2, 1, 3))

    key = (n, heads_per_core, DVE_PAIRS)
    if key not in _PROGRAM_CACHE:
        _PROGRAM_CACHE[key] = _build_program(n, heads_per_core,
                                             frozenset(DVE_PAIRS))
    nc = _PROGRAM_CACHE[key]

    in_maps = []
    for c in range(N_CORES):
        sl = slice(c * heads_per_core, (c + 1) * heads_per_core)
        in_maps.append({"q16": q16[sl], "k16": k16[sl], "v16": v16[sl]})

    trace = bool(int(os.environ.get("BASS_ATTN_TRACE", "0")))
    kwargs = {}
    if trace:
        kwargs = dict(trace=True, trace_cores=list(range(N_CORES)))
    res = run_bass_kernel_spmd(nc, in_maps, core_ids=list(range(N_CORES)),
                               **kwargs)
    LAST_RESULTS = res

    out = np.empty((batch, n, num_heads * dh), np.float32)
    for bh_idx in range(bh):
        c, hh = divmod(bh_idx, heads_per_core)
        b, hd = divmod(bh_idx, num_heads)
        o = res.results[c]["o"][hh]  # [n_ib, 97, IB]
        den = o[:, SUM_ROW, :]  # [n_ib, IB]
        on = o[:, :DH, :] / den[:, None, :]  # [n_ib, 80, IB]
        out[b, :, hd * dh:(hd + 1) * dh] = (
            on.transpose(0, 2, 1).reshape(n, dh))
    return out


# revision 7
# speedup vs baseline: 1.2800x; 1.0084x over previous
"""Masked multi-head self-attention (sparse_attention) on 8 Trainium2 cores.

Strategy (v3)
-------------
Shard the fused (batch*heads)=16 leading dim of q/k/v across 8 cores, 2 heads
per core.  Per head the kernel computes S^T = K''@Q''^T in [j, i] orientation
on the tensor engine, where Q''/K'' carry two extra contraction rows that
encode the bbox mask additively: q''[80]=M*mA_i, q''[81]=M*mB_i and
k''[80]=-M*mB_j, k''[81]=-M*mA_j, so blocked (i,j) pairs get -M^2 added to
the score and fall out of both exp paths naturally.  No key sorting, no
accumulator groups, no combine pass.

Scores are produced pre-scaled into fp16-Schraudolph bit space:
t = A_h*u where u = q.k/sqrt(dh) and A_h = 1024/ln2.  The exp(u - C) of each
[128, 1024] score pair-tile is then evaluated on ONE of TWO engines in
parallel (static assignment):
  - ACT pairs (9/16): scalar-engine exp (scale=1/A_h, bias=-C) -> fp16 P.
  - DVE pairs (7/16): one vector-engine tensor_scalar (add IMM, max 0) ->
    int16 whose bit pattern IS fp16(exp(u-C)) (Schraudolph, ~3% rel err).
Both feed plain fp16 PV matmuls accumulating into one [112, 512] PSUM tile
per i-block; softmax denominators fall out of a ones-column in V.  Per
i-block the accumulator is copied to SBUF and DMA'd out unnormalized; the
host divides by the sums row, transposes, and reassembles heads.  Inputs
stream on two DMA queues (sync: q/k, gpsimd: v + outputs).
"""

import math
import os

import numpy as np

N_CORES = 8
P = 128  # partitions / j-chunk rows
IB = 512  # i-block width (psum bank, fp32)
DH = 80  # head dim
DV = 112  # padded V cols
SUM_ROW = 96
MASK = 192.0  # mask row magnitude; blocked scores get -MASK^2
C_SHIFT = 4.0  # global exp shift (range headroom; cancels in softmax)
MU = 0.044  # Schraudolph bias tuning
A_H = 1024.0 / math.log(2.0)
IMM = 15360.0 - 1024.0 * MU - C_SHIFT * A_H

# pair-tile indices handled by the vector engine (rest go to scalar engine);
# strict alternation keeps each exp engine ahead of the PV matmuls
DVE_PAIRS = (1, 3, 5, 7, 9, 11, 13)

_PROGRAM_CACHE = {}
LAST_RESULTS = None  # BassKernelResults of the most recent run (for test.py)


def _subject_masks_np(bboxes: np.ndarray, resolution: int) -> np.ndarray:
    b = bboxes[0].astype(np.float32)  # [s, 4]
    x0 = np.round(b[:, 0] * resolution)
    y0 = np.round(b[:, 1] * resolution)
    x1 = np.round(b[:, 2] * resolution)
    y1 = np.round(b[:, 3] * resolution)
    coords = np.arange(resolution, dtype=np.float32)
    xm = (coords[None, :] >= x0[:, None]) & (coords[None, :] < x1[:, None])
    ym = (coords[None, :] >= y0[:, None]) & (coords[None, :] < y1[:, None])
    return (ym[:, :, None] & xm[:, None, :]).reshape(b.shape[0], -1)  # [s, n]


def _build_program(n, heads_per_core, dve_pairs):
    import concourse.mybir as mybir
    import concourse.tile as tile
    from concourse import bacc

    f32 = mybir.dt.float32
    f16 = mybir.dt.float16
    i16 = mybir.dt.int16
    Exp = mybir.ActivationFunctionType.Exp
    ADD = mybir.AluOpType.add
    MAX = mybir.AluOpType.max

    nch = n // P
    npair = nch // 2
    n_ib = n // IB

    nc = bacc.Bacc("TRN2", target_bir_lowering=False, debug=False,
                   num_devices=N_CORES)
    q_d = nc.dram_tensor("q16", [heads_per_core, 82, n], f16,
                         kind="ExternalInput")
    k_d = nc.dram_tensor("k16", [heads_per_core, 82, nch, P], f16,
                         kind="ExternalInput")
    v16_d = nc.dram_tensor("v16", [heads_per_core, P, nch, DV], f16,
                           kind="ExternalInput")
    o_d = nc.dram_tensor("o", [heads_per_core, n_ib, 97, IB], f16,
                         kind="ExternalOutput")

    with tile.TileContext(nc) as tc:
        with (
            tc.tile_pool(name="const", bufs=1) as const_pool,
            tc.tile_pool(name="head", bufs=2) as head_pool,
            tc.tile_pool(name="p16", bufs=4) as p16_pool,
            tc.tile_pool(name="pb", bufs=4) as pb_pool,
            tc.tile_pool(name="out", bufs=3) as out_pool,
            tc.tile_pool(name="s_ps", bufs=3, space="PSUM") as s_pool,
            tc.tile_pool(name="acc_ps", bufs=2, space="PSUM") as acc_pool,
        ):
            bias_c = const_pool.tile([P, 1], f32)
            nc.vector.memset(bias_c[:], -C_SHIFT)

            # pre-warm the exp table set while the first DMAs run
            warm = const_pool.tile([P, 1], f32)
            nc.vector.memset(warm[:], 0.0)
            nc.scalar.activation(warm[:], warm[:], Exp)

            def load_head(h):
                q16 = head_pool.tile([82, n], f16, tag="q16", name=f"q16_{h}")
                k16 = head_pool.tile([82, nch, P], f16, tag="k16",
                                     name=f"k16_{h}")
                v16 = head_pool.tile([P, nch, DV], f16, tag="v16",
                                     name=f"v16_{h}")
                # sync queue: q + k, first i-block's needs first (small
                # leading slices so the first matmul starts early)
                nc.sync.dma_start(q16[:, 0:IB], q_d[h][:, 0:IB])
                nc.sync.dma_start(k16[:, 0:2, :], k_d[h][:, 0:2, :])
                nc.sync.dma_start(k16[:, 2:8, :], k_d[h][:, 2:8, :])
                nc.sync.dma_start(k16[:, 8:20, :], k_d[h][:, 8:20, :])
                nc.sync.dma_start(k16[:, 20:nch, :], k_d[h][:, 20:nch, :])
                nc.sync.dma_start(q16[:, IB:n], q_d[h][:, IB:n])
                # gpsimd queue: v
                nc.gpsimd.dma_start(v16[:, 0:2, :], v16_d[h][:, 0:2, :])
                nc.gpsimd.dma_start(v16[:, 2:8, :], v16_d[h][:, 2:8, :])
                nc.gpsimd.dma_start(v16[:, 8:20, :], v16_d[h][:, 8:20, :])
                nc.gpsimd.dma_start(v16[:, 20:nch, :], v16_d[h][:, 20:nch, :])
                return q16, k16, v16

            head_tiles = {0: load_head(0)}
            pending_pv = None
            pending_out = None

            for h in range(heads_per_core):
                if h not in head_tiles:
                    head_tiles[h] = load_head(h)
                if h + 1 < heads_per_core and h + 1 not in head_tiles:
                    head_tiles[h + 1] = load_head(h + 1)
                q16, k16, v16 = head_tiles[h]

                for ib in range(n_ib):
                    acc = acc_pool.tile([DV, IB], f32, tag="acc",
                                        name=f"acc_{h}_{ib}")
                    q_sl = q16[:, ib * IB:(ib + 1) * IB]
                    first = [True]

                    for t in range(npair):
                        c0, c1 = 2 * t, 2 * t + 1
                        s = s_pool.tile([P, 2 * IB], f32, tag="s")
                        nc.tensor.matmul(s[:, 0:IB], lhsT=k16[:, c0, :],
                                         rhs=q_sl, start=True, stop=True)
                        nc.tensor.matmul(s[:, IB:2 * IB], lhsT=k16[:, c1, :],
                                         rhs=q_sl, start=True, stop=True)

                        if t in dve_pairs:
                            pb = pb_pool.tile([P, 2 * IB], i16, tag="pb")
                            nc.vector.tensor_scalar(pb[:], s[:], IMM, 0.0,
                                                    op0=ADD, op1=MAX)
                            rhs0 = pb[:, 0:IB].bitcast(f16)
                            rhs1 = pb[:, IB:2 * IB].bitcast(f16)
                        else:
                            p16 = p16_pool.tile([P, 2 * IB], f16, tag="p16")
                            nc.scalar.activation(p16[:], s[:], Exp,
                                                 scale=float(1.0 / A_H),
                                                 bias=bias_c[:])
                            rhs0 = p16[:, 0:IB]
                            rhs1 = p16[:, IB:2 * IB]

                        def make_pv(rhs0=rhs0, rhs1=rhs1, c0=c0, c1=c1, t=t,
                                    acc=acc, v16=v16, first=first):
                            def pv():
                                nc.tensor.matmul(
                                    acc[:], lhsT=v16[:, c0, :], rhs=rhs0,
                                    start=first[0], stop=False)
                                first[0] = False
                                nc.tensor.matmul(
                                    acc[:], lhsT=v16[:, c1, :], rhs=rhs1,
                                    start=False, stop=(t == npair - 1))
                            return pv

                        if pending_pv is not None:
                            pending_pv()
                        pending_pv = make_pv()
                        if t == 2 and pending_out is not None:
                            pending_out()
                            pending_out = None

                    def make_out(acc=acc, h=h, ib=ib):
                        def out():
                            o_sb = out_pool.tile([97, IB], f16, tag="osb",
                                                 name=f"o_{h}_{ib}")
                            nc.vector.tensor_copy(o_sb[:], acc[0:97, :])
                            nc.sync.dma_start(o_d[h, ib], o_sb[:])
                        return out

                    if pending_out is not None:
                        pending_out()
                    pending_out = make_out()

            if pending_pv is not None:
                pending_pv()
            if pending_out is not None:
                pending_out()

    nc.compile()
    return nc


def kernel(hidden_states, q, k, v, bboxes, is_cross, ith, num_heads):
    global LAST_RESULTS
    if is_cross:
        return np.asarray(hidden_states)

    from concourse.bass_utils import run_bass_kernel_spmd

    q = np.asarray(q, dtype=np.float32)
    k = np.asarray(k, dtype=np.float32)
    v = np.asarray(v, dtype=np.float32)
    bboxes = np.asarray(bboxes, dtype=np.float32)
    num_heads = int(num_heads)

    bh, n, dh = q.shape
    assert dh == DH and bh % N_CORES == 0 and n % IB == 0
    heads_per_core = bh // N_CORES
    batch = bh // num_heads
    nch = n // P
    n_ib = n // IB
    scale = 1.0 / math.sqrt(dh)
    g = math.sqrt(A_H * scale)

    res_sq = int(math.isqrt(n))
    subj = _subject_masks_np(bboxes, res_sq)
    assert subj.shape[0] == 2, "kernel specialized for 2 subject boxes"
    mA = (subj[0] & ~subj[1]).astype(np.float32)  # A-only
    mB = (subj[1] & ~subj[0]).astype(np.float32)  # B-only

    f16 = np.float16

    qT = q.transpose(0, 2, 1) * g  # [bh, 80, n]
    kT = k.transpose(0, 2, 1) * g
    q16 = np.empty((bh, 82, n), f16)
    q16[:, :DH] = qT
    q16[:, DH] = MASK * mA
    q16[:, DH + 1] = MASK * mB
    k16 = np.empty((bh, 82, n), f16)
    k16[:, :DH] = kT
    k16[:, DH] = -MASK * mB
    k16[:, DH + 1] = -MASK * mA
    k16 = k16.reshape(bh, 82, nch, P)

    vt = np.zeros((bh, n, DV), f16)
    vt[:, :, :DH] = v
    vt[:, :, SUM_ROW] = 1.0
    v16 = np.ascontiguousarray(
        vt.reshape(bh, nch, P, DV).transpose(0, 2, 1, 3))

    key = (n, heads_per_core, DVE_PAIRS)
    if key not in _PROGRAM_CACHE:
        _PROGRAM_CACHE[key] = _build_program(n, heads_per_core,
                                             frozenset(DVE_PAIRS))
    nc = _PROGRAM_CACHE[key]

    in_maps = []
    for c in range(N_CORES):
        sl = slice(c * heads_per_core, (c + 1) * heads_per_core)
        in_maps.append({"q16": q16[sl], "k16": k16[sl], "v16": v16[sl]})

    trace = bool(int(os.environ.get("BASS_ATTN_TRACE", "0")))
    kwargs = {}
    if trace:
        kwargs = dict(trace=True, trace_cores=list(range(N_CORES)))
    res = run_bass_kernel_spmd(nc, in_maps, core_ids=list(range(N_CORES)),
                               **kwargs)
    LAST_RESULTS = res

    out = np.empty((batch, n, num_heads * dh), np.float32)
    for bh_idx in range(bh):
        c, hh = divmod(bh_idx, heads_per_core)
        b, hd = divmod(bh_idx, num_heads)
        o = res.results[c]["o"][hh].astype(np.float32)  # [n_ib, 97, IB]
        den = o[:, SUM_ROW, :]  # [n_ib, IB]
        on = o[:, :DH, :] / den[:, None, :]  # [n_ib, 80, IB]
        out[b, :, hd * dh:(hd + 1) * dh] = (
            on.transpose(0, 2, 1).reshape(n, dh))
    return out


# revision 10
# speedup vs baseline: 1.2828x; 1.0022x over previous
"""Masked multi-head self-attention (sparse_attention) on 8 Trainium2 cores.

Strategy (v3)
-------------
Shard the fused (batch*heads)=16 leading dim of q/k/v across 8 cores, 2 heads
per core.  Per head the kernel computes S^T = K''@Q''^T in [j, i] orientation
on the tensor engine, where Q''/K'' carry two extra contraction rows that
encode the bbox mask additively: q''[80]=M*mA_i, q''[81]=M*mB_i and
k''[80]=-M*mB_j, k''[81]=-M*mA_j, so blocked (i,j) pairs get -M^2 added to
the score and fall out of both exp paths naturally.  No key sorting, no
accumulator groups, no combine pass.

Scores are produced pre-scaled into fp16-Schraudolph bit space:
t = A_h*u where u = q.k/sqrt(dh) and A_h = 1024/ln2.  The exp(u - C) of each
[128, 1024] score pair-tile is then evaluated on ONE of TWO engines in
parallel (static assignment):
  - ACT pairs (9/16): scalar-engine exp (scale=1/A_h, bias=-C) -> fp16 P.
  - DVE pairs (7/16): one vector-engine tensor_scalar (add IMM, max 0) ->
    int16 whose bit pattern IS fp16(exp(u-C)) (Schraudolph, ~3% rel err).
Both feed plain fp16 PV matmuls accumulating into one [112, 512] PSUM tile
per i-block; softmax denominators fall out of a ones-column in V.  Per
i-block the accumulator is copied to SBUF and DMA'd out unnormalized; the
host divides by the sums row, transposes, and reassembles heads.  Inputs
stream on two DMA queues (sync: q/k, gpsimd: v + outputs).
"""

import math
import os

import numpy as np

N_CORES = 8
P = 128  # partitions / j-chunk rows
IB = 512  # i-block width (psum bank, fp32)
DH = 80  # head dim
DV = 112  # padded V cols
SUM_ROW = 96
MASK = 192.0  # mask row magnitude; blocked scores get -MASK^2
C_SHIFT = 4.0  # global exp shift (range headroom; cancels in softmax)
MU = 0.044  # Schraudolph bias tuning
A_H = 1024.0 / math.log(2.0)
IMM = 15360.0 - 1024.0 * MU - C_SHIFT * A_H

# pair-tile indices handled by the vector engine (rest go to scalar engine);
# strict alternation keeps each exp engine ahead of the PV matmuls
DVE_PAIRS = (1, 3, 5, 7, 9, 11, 13)

_PROGRAM_CACHE = {}
LAST_RESULTS = None  # BassKernelResults of the most recent run (for test.py)


def _subject_masks_np(bboxes: np.ndarray, resolution: int) -> np.ndarray:
    b = bboxes[0].astype(np.float32)  # [s, 4]
    x0 = np.round(b[:, 0] * resolution)
    y0 = np.round(b[:, 1] * resolution)
    x1 = np.round(b[:, 2] * resolution)
    y1 = np.round(b[:, 3] * resolution)
    coords = np.arange(resolution, dtype=np.float32)
    xm = (coords[None, :] >= x0[:, None]) & (coords[None, :] < x1[:, None])
    ym = (coords[None, :] >= y0[:, None]) & (coords[None, :] < y1[:, None])
    return (ym[:, :, None] & xm[:, None, :]).reshape(b.shape[0], -1)  # [s, n]


def _build_program(n, heads_per_core, dve_pairs):
    import concourse.mybir as mybir
    import concourse.tile as tile
    from concourse import bacc

    f32 = mybir.dt.float32
    f16 = mybir.dt.float16
    i16 = mybir.dt.int16
    Exp = mybir.ActivationFunctionType.Exp
    ADD = mybir.AluOpType.add
    MAX = mybir.AluOpType.max

    nch = n // P
    npair = nch // 2
    n_ib = n // IB

    nc = bacc.Bacc("TRN2", target_bir_lowering=False, debug=False,
                   num_devices=N_CORES)
    q_d = nc.dram_tensor("q16", [heads_per_core, 82, n], f16,
                         kind="ExternalInput")
    k_d = nc.dram_tensor("k16", [heads_per_core, 82, nch, P], f16,
                         kind="ExternalInput")
    v16_d = nc.dram_tensor("v16", [heads_per_core, P, nch, DV], f16,
                           kind="ExternalInput")
    o_d = nc.dram_tensor("o", [heads_per_core, n_ib, 97, IB], f16,
                         kind="ExternalOutput")

    with tile.TileContext(nc) as tc:
        with (
            tc.tile_pool(name="const", bufs=1) as const_pool,
            tc.tile_pool(name="head", bufs=2) as head_pool,
            tc.tile_pool(name="p16", bufs=4) as p16_pool,
            tc.tile_pool(name="pb", bufs=4) as pb_pool,
            tc.tile_pool(name="out", bufs=3) as out_pool,
            tc.tile_pool(name="s_ps", bufs=3, space="PSUM") as s_pool,
            tc.tile_pool(name="acc_ps", bufs=2, space="PSUM") as acc_pool,
        ):
            bias_c = const_pool.tile([P, 1], f32)
            nc.vector.memset(bias_c[:], -C_SHIFT)

            # pre-warm the exp table set while the first DMAs run
            warm = const_pool.tile([P, 1], f32)
            nc.vector.memset(warm[:], 0.0)
            nc.scalar.activation(warm[:], warm[:], Exp)

            # PE warm-up: dependency-free matmuls on const data during the
            # DMA ramp so the HAM clock gate reaches 8/8 before real work
            warm_w = const_pool.tile([P, 256], f16)
            nc.vector.memset(warm_w[:], 0.0)
            for r in range(24):
                sw = s_pool.tile([P, 2 * IB], f32, tag="s", name=f"warm{r}")
                nc.tensor.matmul(sw[:, 0:256], lhsT=warm_w[:, 0:P],
                                 rhs=warm_w[:], start=True, stop=True)

            def load_head(h, first):
                q16 = head_pool.tile([82, n], f16, tag="q16", name=f"q16_{h}")
                k16 = head_pool.tile([82, nch, P], f16, tag="k16",
                                     name=f"k16_{h}")
                v16 = head_pool.tile([P, nch, DV], f16, tag="v16",
                                     name=f"v16_{h}")
                if first:
                    # head 0: spread across three queues so the first
                    # i-blocks' operands land as early as possible
                    nc.scalar.dma_start(k16[:, 0:2, :], k_d[h][:, 0:2, :])
                    nc.sync.dma_start(q16[:, 0:IB], q_d[h][:, 0:IB])
                    nc.scalar.dma_start(k16[:, 2:8, :], k_d[h][:, 2:8, :])
                    nc.gpsimd.dma_start(v16[:, 0:8, :], v16_d[h][:, 0:8, :])
                    nc.sync.dma_start(k16[:, 8:20, :], k_d[h][:, 8:20, :])
                    nc.gpsimd.dma_start(v16[:, 8:20, :],
                                        v16_d[h][:, 8:20, :])
                    nc.sync.dma_start(q16[:, IB:2 * IB],
                                      q_d[h][:, IB:2 * IB])
                    nc.gpsimd.dma_start(k16[:, 20:nch, :],
                                        k_d[h][:, 20:nch, :])
                    nc.sync.dma_start(q16[:, 2 * IB:4 * IB],
                                      q_d[h][:, 2 * IB:4 * IB])
                    nc.gpsimd.dma_start(v16[:, 20:nch, :],
                                        v16_d[h][:, 20:nch, :])
                    nc.sync.dma_start(q16[:, 4 * IB:n], q_d[h][:, 4 * IB:n])
                else:
                    # prefetch: stay off the scalar/vector queues
                    nc.sync.dma_start(q16[:], q_d[h][:])
                    nc.gpsimd.dma_start(k16[:], k_d[h][:])
                    nc.gpsimd.dma_start(v16[:], v16_d[h][:])
                return q16, k16, v16

            head_tiles = {0: load_head(0, True)}
            pending_pv = None
            pending_out = None

            for h in range(heads_per_core):
                if h not in head_tiles:
                    head_tiles[h] = load_head(h, False)
                q16, k16, v16 = head_tiles[h]

                for ib in range(n_ib):
                    if ib == 3 and h + 1 < heads_per_core \
                            and h + 1 not in head_tiles:
                        head_tiles[h + 1] = load_head(h + 1, False)
                    acc = acc_pool.tile([DV, IB], f32, tag="acc",
                                        name=f"acc_{h}_{ib}")
                    q_sl = q16[:, ib * IB:(ib + 1) * IB]
                    first = [True]

                    for t in range(npair):
                        c0, c1 = 2 * t, 2 * t + 1
                        s = s_pool.tile([P, 2 * IB], f32, tag="s")
                        nc.tensor.matmul(s[:, 0:IB], lhsT=k16[:, c0, :],
                                         rhs=q_sl, start=True, stop=True)
                        nc.tensor.matmul(s[:, IB:2 * IB], lhsT=k16[:, c1, :],
                                         rhs=q_sl, start=True, stop=True)

                        if t in dve_pairs:
                            pb = pb_pool.tile([P, 2 * IB], i16, tag="pb")
                            nc.vector.tensor_scalar(pb[:], s[:], IMM, 0.0,
                                                    op0=ADD, op1=MAX)
                            rhs0 = pb[:, 0:IB].bitcast(f16)
                            rhs1 = pb[:, IB:2 * IB].bitcast(f16)
                        else:
                            p16 = p16_pool.tile([P, 2 * IB], f16, tag="p16")
                            nc.scalar.activation(p16[:], s[:], Exp,
                                                 scale=float(1.0 / A_H),
                                                 bias=bias_c[:])
                            rhs0 = p16[:, 0:IB]
                            rhs1 = p16[:, IB:2 * IB]

                        def make_pv(rhs0=rhs0, rhs1=rhs1, c0=c0, c1=c1, t=t,
                                    acc=acc, v16=v16, first=first):
                            def pv():
                                nc.tensor.matmul(
                                    acc[:], lhsT=v16[:, c0, :], rhs=rhs0,
                                    start=first[0], stop=False)
                                first[0] = False
                                nc.tensor.matmul(
                                    acc[:], lhsT=v16[:, c1, :], rhs=rhs1,
                                    start=False, stop=(t == npair - 1))
                            return pv

                        if pending_pv is not None:
                            pending_pv()
                        pending_pv = make_pv()
                        if t == 2 and pending_out is not None:
                            pending_out()
                            pending_out = None

                    def make_out(acc=acc, h=h, ib=ib):
                        def out():
                            o_sb = out_pool.tile([97, IB], f16, tag="osb",
                                                 name=f"o_{h}_{ib}")
                            nc.vector.tensor_copy(o_sb[:], acc[0:97, :])
                            eng = nc.sync if ib % 2 == 0 else nc.gpsimd
                            eng.dma_start(o_d[h, ib], o_sb[:])
                        return out

                    if pending_out is not None:
                        pending_out()
                    pending_out = make_out()

            if pending_pv is not None:
                pending_pv()
            if pending_out is not None:
                pending_out()

    nc.compile()
    return nc


def kernel(hidden_states, q, k, v, bboxes, is_cross, ith, num_heads):
    global LAST_RESULTS
    if is_cross:
        return np.asarray(hidden_states)

    from concourse.bass_utils import run_bass_kernel_spmd

    q = np.asarray(q, dtype=np.float32)
    k = np.asarray(k, dtype=np.float32)
    v = np.asarray(v, dtype=np.float32)
    bboxes = np.asarray(bboxes, dtype=np.float32)
    num_heads = int(num_heads)

    bh, n, dh = q.shape
    assert dh == DH and bh % N_CORES == 0 and n % IB == 0
    heads_per_core = bh // N_CORES
    batch = bh // num_heads
    nch = n // P
    n_ib = n // IB
    scale = 1.0 / math.sqrt(dh)
    g = math.sqrt(A_H * scale)

    res_sq = int(math.isqrt(n))
    subj = _subject_masks_np(bboxes, res_sq)
    assert subj.shape[0] == 2, "kernel specialized for 2 subject boxes"
    mA = (subj[0] & ~subj[1]).astype(np.float32)  # A-only
    mB = (subj[1] & ~subj[0]).astype(np.float32)  # B-only

    f16 = np.float16

    qT = q.transpose(0, 2, 1) * g  # [bh, 80, n]
    kT = k.transpose(0, 2, 1) * g
    q16 = np.empty((bh, 82, n), f16)
    q16[:, :DH] = qT
    q16[:, DH] = MASK * mA
    q16[:, DH + 1] = MASK * mB
    k16 = np.empty((bh, 82, n), f16)
    k16[:, :DH] = kT
    k16[:, DH] = -MASK * mB
    k16[:, DH + 1] = -MASK * mA
    k16 = k16.reshape(bh, 82, nch, P)

    vt = np.zeros((bh, n, DV), f16)
    vt[:, :, :DH] = v
    vt[:, :, SUM_ROW] = 1.0
    v16 = np.ascontiguousarray(
        vt.reshape(bh, nch, P, DV).transpose(0, 2, 1, 3))

    key = (n, heads_per_core, DVE_PAIRS)
    if key not in _PROGRAM_CACHE:
        _PROGRAM_CACHE[key] = _build_program(n, heads_per_core,
                                             frozenset(DVE_PAIRS))
    nc = _PROGRAM_CACHE[key]

    in_maps = []
    for c in range(N_CORES):
        sl = slice(c * heads_per_core, (c + 1) * heads_per_core)
        in_maps.append({"q16": q16[sl], "k16": k16[sl], "v16": v16[sl]})

    trace = bool(int(os.environ.get("BASS_ATTN_TRACE", "0")))
    kwargs = {}
    if trace:
        kwargs = dict(trace=True, trace_cores=list(range(N_CORES)))
    res = run_bass_kernel_spmd(nc, in_maps, core_ids=list(range(N_CORES)),
                               **kwargs)
    LAST_RESULTS = res

    out = np.empty((batch, n, num_heads * dh), np.float32)
    for bh_idx in range(bh):
        c, hh = divmod(bh_idx, heads_per_core)
        b, hd = divmod(bh_idx, num_heads)
        o = res.results[c]["o"][hh].astype(np.float32)  # [n_ib, 97, IB]
        den = o[:, SUM_ROW, :]  # [n_ib, IB]
        on = o[:, :DH, :] / den[:, None, :]  # [n_ib, 80, IB]
        out[b, :, hd * dh:(hd + 1) * dh] = (
            on.transpose(0, 2, 1).reshape(n, dh))
    return out
